# revision 1
# baseline (speedup 1.0000x reference)
"""Trainium2 Bass kernel for a linear-attention block (ELU+1 feature map).

Computation (per batch b):
  Q = elu(query @ Wq + bq) + 1 ; K = elu(key @ Wk + bk) + 1 ; V = value @ Wv + bv
  out[t] = Q[t] * cumsum_excl(K*V)[t] / (sum_{d in head}(Q[t]*cumsum_excl(K)[t]) + eps)
  attn = out @ Wo + bo ;  y = LayerNorm(query + attn) * gamma + beta

Sharding: 8 cores = (batch b in 0..3) x (L-half h in 0..1); each core owns 2048
contiguous rows of one batch.  Two SPMD launches:
  L1: QKV projections (bf16 matmuls; transposed activation layout: channels on
      partitions, tokens on free dim), feature map (elu(x)+1 = min(exp(x),1)
      + relu(x)), K*V, local exclusive cumsums via tensor_tensor_scan,
      per-channel totals.
  host: totals -> per-core cumsum offsets, pre-added into the spilled cumsums;
      bo folded into the query rows.
  L2: attention math + Wo projection (back to natural token-row layout) +
      residual + LayerNorm.
"""

import sys

if "/opt/trn_rl_repo" not in sys.path:
    sys.path.insert(0, "/opt/trn_rl_repo")

import numpy as np
import ml_dtypes

import concourse.bass as bass
import concourse.mybir as mybir
import concourse.tile as tile
import concourse.bass_utils as bass_utils
import concourse.bass2jax as bass2jax
from concourse.bass_utils import run_bass_kernel_spmd


# --------------------------------------------------------------------------
# Compile fix: the walrus build in this container rejects instructions whose
# sync_info carries more than one on_wait ("Too many sync wait commands").
# Tile attaches multi-wait sync_info; split the extras into standalone
# EventSemaphore instructions (exactly what raw bass emits for wait_ge),
# which this walrus accepts.  Semantics preserved: engines are in-order, so
# waiting before the instruction == waiting on the instruction.
# --------------------------------------------------------------------------
def _split_multi_waits(bir_json):
    import json as _json

    bir = _json.loads(bir_json)
    ctr = 0
    changed = False
    for fn in bir.get("functions", []):
        for blk in fn.get("blocks", []):
            out = []
            for inst in blk.get("instructions", []):
                si = inst.get("sync_info")
                waits = (si or {}).get("on_wait") or []
                if len(waits) > 1:
                    for w in waits[:-1]:
                        ctr += 1
                        out.append({
                            "name": f"EVSx-{ctr}",
                            "opcode": "EventSemaphore",
                            "engine": inst["engine"],
                            "ins": [], "outs": [],
                            "sync_info": {"on_update": [], "on_wait": [w]},
                        })
                    si["on_wait"] = waits[-1:]
                    changed = True
                out.append(inst)
            blk["instructions"] = out
    if not changed:
        return bir_json
    return _json.dumps(bir).encode()


_orig_compile_bir_kernel = bass_utils.compile_bir_kernel


def _compile_bir_kernel_splitwaits(bir_json, tmpdir, neff_name="file.neff"):
    return _orig_compile_bir_kernel(_split_multi_waits(bir_json), tmpdir, neff_name)


if getattr(bass_utils.compile_bir_kernel, "__name__", "") != (
    "_compile_bir_kernel_splitwaits"
):
    bass_utils.compile_bir_kernel = _compile_bir_kernel_splitwaits
    bass2jax.compile_bir_kernel = _compile_bir_kernel_splitwaits

BF16 = ml_dtypes.bfloat16
F32 = np.float32

B, L, DM, H, D = 4, 4096, 1024, 16, 64
NCORES = 8
LH = L // 2          # 2048 rows per core
P = 128              # partitions
NCH = DM // P        # 8 channel chunks of 128
HPC = P // D         # 2 heads per channel chunk
TB = 512             # token block (matmul free dim)
NTB = LH // TB       # 4 token blocks per core
EPS_ATTN = 1e-9
EPS_LN = 1e-6

_FP = mybir.dt.float32
_BF = mybir.dt.bfloat16
_ALU = mybir.AluOpType
_ACTF = mybir.ActivationFunctionType

# toggles for test harness
TRACE = False
LAST_PROFILE = {}


# --------------------------------------------------------------------------
# Launch 1: projections + feature map + local exclusive cumsums
# --------------------------------------------------------------------------
def build_l1():
    nc = bass.Bass(name="linattn_l1")
    qT = nc.dram_tensor("qT", [DM, LH], _BF, kind="ExternalInput")
    kT = nc.dram_tensor("kT", [DM, LH], _BF, kind="ExternalInput")
    vT = nc.dram_tensor("vT", [DM, LH], _BF, kind="ExternalInput")
    wq = nc.dram_tensor("wq", [P, NCH, DM], _BF, kind="ExternalInput")
    wk = nc.dram_tensor("wk", [P, NCH, DM], _BF, kind="ExternalInput")
    wv = nc.dram_tensor("wv", [P, NCH, DM], _BF, kind="ExternalInput")
    bqkv = nc.dram_tensor("bqkv", [P, 3 * NCH], _FP, kind="ExternalInput")

    qf = nc.dram_tensor("qf", [DM, LH], _BF, kind="ExternalOutput")
    sk = nc.dram_tensor("sk", [DM, LH], _BF, kind="ExternalOutput")
    skv = nc.dram_tensor("skv", [DM, LH], _BF, kind="ExternalOutput")
    tot = nc.dram_tensor("tot", [P, 2 * NCH], _FP, kind="ExternalOutput")

    x_view = {
        "q": qT.rearrange("(o p) t -> p o t", p=P),
        "k": kT.rearrange("(o p) t -> p o t", p=P),
        "v": vT.rearrange("(o p) t -> p o t", p=P),
    }
    t_dram = {"q": wq, "k": wk, "v": wv}
    qf_view = qf.rearrange("(o p) t -> p o t", p=P)
    sk_view = sk.rearrange("(o p) t -> p o t", p=P)
    skv_view = skv.rearrange("(o p) t -> p o t", p=P)

    with tile.TileContext(nc) as tc:
        with (
            tc.tile_pool(name="wpool", bufs=1) as wpool,
            tc.tile_pool(name="xpool", bufs=1) as xpool,
            tc.tile_pool(name="cpool", bufs=1) as cpool,
            tc.tile_pool(name="fmap", bufs=2) as fmap,
            tc.tile_pool(name="rows", bufs=2) as rows,
            tc.tile_pool(name="srows", bufs=1) as srows,
            tc.tile_pool(name="ps", bufs=2, space="PSUM") as ps,
        ):
            # constants / weights
            w_sb = {}
            for name, t in (("q", wq), ("k", wk), ("v", wv)):
                w_sb[name] = wpool.tile(
                    [P, NCH, DM], _BF, tag=f"w{name}", name=f"w{name}"
                )
                nc.sync.dma_start(w_sb[name][:], t[:])
            bias_sb = cpool.tile([P, 3 * NCH], _FP, tag="bias")
            nc.sync.dma_start(bias_sb[:], bqkv[:])

            # activations: full (P, NCH, LH) per tensor, loaded interleaved in
            # t-slices so the first (ci=0, tb=0) matmuls can start early
            x_sb = {}
            for name in ("q", "k", "v"):
                x_sb[name] = xpool.tile(
                    [P, NCH, LH], _BF, tag=f"x{name}", name=f"x{name}"
                )
            for tb in range(NTB):
                tsl = slice(tb * TB, (tb + 1) * TB)
                for name in ("q", "k", "v"):
                    nc.sync.dma_start(x_sb[name][:, :, tsl], x_view[name][:, :, tsl])

            tot_tile = cpool.tile([P, 2 * NCH], _FP, tag="tot")
            kcar7 = cpool.tile([P, 1], _FP, tag="kcar7")
            kvcar7 = cpool.tile([P, 1], _FP, tag="kvcar7")
            nc.vector.memset(kcar7[:], 0.0)
            nc.vector.memset(kvcar7[:], 0.0)

            for ci in range(NCH):
                csl = slice(ci * P, (ci + 1) * P)
                kbuf = rows.tile([P, LH], _BF, tag="kbuf")
                kvbuf = rows.tile([P, LH], _BF, tag="kvbuf")
                qfbuf = rows.tile([P, LH], _BF, tag="qfbuf")

                for tb in range(NTB):
                    tsl = slice(tb * TB, (tb + 1) * TB)
                    ps_q = ps.tile([P, TB], _FP, tag="psq")
                    ps_k = ps.tile([P, TB], _FP, tag="psk")
                    ps_v = ps.tile([P, TB], _FP, tag="psv")
                    for o in range(NCH):
                        nc.tensor.matmul(
                            ps_q, w_sb["q"][:, o, csl], x_sb["q"][:, o, tsl],
                            start=(o == 0), stop=(o == NCH - 1),
                        )
                    for o in range(NCH):
                        nc.tensor.matmul(
                            ps_k, w_sb["k"][:, o, csl], x_sb["k"][:, o, tsl],
                            start=(o == 0), stop=(o == NCH - 1),
                        )
                    for o in range(NCH):
                        nc.tensor.matmul(
                            ps_v, w_sb["v"][:, o, csl], x_sb["v"][:, o, tsl],
                            start=(o == 0), stop=(o == NCH - 1),
                        )

                    # q' = min(exp(qlin+bq), 1) + relu(qlin+bq)
                    e_t = fmap.tile([P, TB], _BF, tag="e")
                    r_t = fmap.tile([P, TB], _BF, tag="r")
                    qb = bias_sb[:, ci:ci + 1]
                    nc.scalar.activation(e_t[:], ps_q[:], _ACTF.Exp, bias=qb)
                    nc.scalar.activation(r_t[:], ps_q[:], _ACTF.Relu, bias=qb)
                    nc.vector.scalar_tensor_tensor(
                        qfbuf[:, tsl], e_t[:], 1.0, r_t[:], _ALU.min, _ALU.add
                    )

                    # k' into kbuf
                    ek_t = fmap.tile([P, TB], _BF, tag="ek")
                    rk_t = fmap.tile([P, TB], _BF, tag="rk")
                    kb = bias_sb[:, NCH + ci:NCH + ci + 1]
                    nc.scalar.activation(ek_t[:], ps_k[:], _ACTF.Exp, bias=kb)
                    nc.scalar.activation(rk_t[:], ps_k[:], _ACTF.Relu, bias=kb)
                    nc.vector.scalar_tensor_tensor(
                        kbuf[:, tsl], ek_t[:], 1.0, rk_t[:], _ALU.min, _ALU.add
                    )
                    # kv = (vlin + bv) * k'
                    vb = bias_sb[:, 2 * NCH + ci:2 * NCH + ci + 1]
                    nc.vector.scalar_tensor_tensor(
                        kvbuf[:, tsl], ps_v[:], vb, kbuf[:, tsl],
                        _ALU.add, _ALU.mult,
                    )

                    if ci == NCH - 1:
                        # last chunk: chained per-block scans so the cumsums
                        # overlap this chunk's own matmuls instead of
                        # trailing the whole kernel
                        for nm, buf, car in (
                            ("sk", kbuf, kcar7), ("skv", kvbuf, kvcar7)
                        ):
                            sbt = srows.tile(
                                [P, TB + 2], _BF, tag=f"c{nm}", name=f"c{nm}"
                            )
                            nc.vector.tensor_copy(sbt[:, 1:2], car[:, 0:1])
                            nc.vector.tensor_tensor_scan(
                                sbt[:, 2:TB + 2], buf[:, tsl], buf[:, tsl],
                                car[:, 0:1], _ALU.add, _ALU.bypass,
                            )
                            view = sk_view if nm == "sk" else skv_view
                            nc.sync.dma_start(
                                view[:, ci, tsl], sbt[:, 1:TB + 1]
                            )
                            nc.vector.tensor_copy(
                                car[:, 0:1], sbt[:, TB + 1:TB + 2]
                            )
                        nc.sync.dma_start(qf_view[:, ci, tsl], qfbuf[:, tsl])

                if ci == NCH - 1:
                    nc.vector.tensor_copy(tot_tile[:, ci:ci + 1], kcar7[:, 0:1])
                    nc.vector.tensor_copy(
                        tot_tile[:, NCH + ci:NCH + ci + 1], kvcar7[:, 0:1]
                    )
                    continue
                nc.sync.dma_start(qf_view[:, ci, :], qfbuf[:])

                # inclusive cumsum into [2:], then spill the exclusive view
                # [1:LH+1]; the inclusive total sits at [LH+1].
                skb = srows.tile([P, LH + 2], _BF, tag="skb")
                skvb = srows.tile([P, LH + 2], _BF, tag="skvb")
                nc.vector.memset(skb[:, 0:2], 0.0)
                nc.vector.memset(skvb[:, 0:2], 0.0)
                nc.vector.tensor_tensor_scan(
                    skb[:, 2:LH + 2], kbuf[:], kbuf[:], 0.0, _ALU.add, _ALU.bypass
                )
                nc.vector.tensor_tensor_scan(
                    skvb[:, 2:LH + 2], kvbuf[:], kvbuf[:], 0.0,
                    _ALU.add, _ALU.bypass,
                )
                nc.sync.dma_start(sk_view[:, ci, :], skb[:, 1:LH + 1])
                nc.sync.dma_start(skv_view[:, ci, :], skvb[:, 1:LH + 1])
                nc.vector.tensor_copy(
                    tot_tile[:, ci:ci + 1], skb[:, LH + 1:LH + 2]
                )
                nc.vector.tensor_copy(
                    tot_tile[:, NCH + ci:NCH + ci + 1], skvb[:, LH + 1:LH + 2]
                )
            nc.sync.dma_start(tot[:], tot_tile[:])
    return nc


# --------------------------------------------------------------------------
# Launch 2: attention math + Wo projection + residual + LayerNorm
# (offsets and bo are folded in on the host; gamma/beta handled on-device
# only when non-trivial)
# --------------------------------------------------------------------------
def build_l2(trivial_gb):
    nc = bass.Bass(name="linattn_l2")
    qf = nc.dram_tensor("qf", [DM, LH], _BF, kind="ExternalInput")
    sk = nc.dram_tensor("sk", [DM, LH], _BF, kind="ExternalInput")
    skv = nc.dram_tensor("skv", [DM, LH], _BF, kind="ExternalInput")
    qrows = nc.dram_tensor("qrows", [LH, DM], _FP, kind="ExternalInput")
    wo = nc.dram_tensor("wo", [P, NCH, DM], _BF, kind="ExternalInput")
    hm = nc.dram_tensor("hm", [P, NCH, H], _BF, kind="ExternalInput")
    hmT = nc.dram_tensor("hmT", [H, NCH, P], _BF, kind="ExternalInput")
    if not trivial_gb:
        gb = nc.dram_tensor("gb", [2, DM], _FP, kind="ExternalInput")

    out = nc.dram_tensor("out", [LH, DM], _FP, kind="ExternalOutput")

    qf_view = qf.rearrange("(o p) t -> p o t", p=P)
    sk_view = sk.rearrange("(o p) t -> p o t", p=P)
    skv_view = skv.rearrange("(o p) t -> p o t", p=P)

    with tile.TileContext(nc) as tc:
        with (
            tc.tile_pool(name="cpool", bufs=1) as cpool,
            tc.tile_pool(name="xin", bufs=2) as xin,
            tc.tile_pool(name="att", bufs=3) as att,
            tc.tile_pool(name="apool", bufs=2) as apool,
            tc.tile_pool(name="ops", bufs=6) as ops,
            tc.tile_pool(name="psdn", bufs=2, space="PSUM") as psdn,
            tc.tile_pool(name="psrep", bufs=2, space="PSUM") as psrep,
            tc.tile_pool(name="psao", bufs=3, space="PSUM") as psao,
        ):
            wo_sb = cpool.tile([P, NCH, DM], _BF, tag="wo")
            nc.sync.dma_start(wo_sb[:], wo[:])
            hm_sb = cpool.tile([P, NCH, H], _BF, tag="hm")
            nc.sync.dma_start(hm_sb[:], hm[:])
            hmT_sb = cpool.tile([H, NCH, P], _BF, tag="hmT")
            nc.sync.dma_start(hmT_sb[:], hmT[:])
            eps_sb = cpool.tile([P, 1], _FP, tag="eps")
            nc.vector.memset(eps_sb[:], EPS_LN)
            if not trivial_gb:
                gamma_rep = cpool.tile([P, DM], _FP, tag="gamma")
                nc.sync.dma_start(gamma_rep[:], gb[0:1, :].to_broadcast([P, DM]))
                beta_rep = cpool.tile([P, DM], _FP, tag="beta")
                nc.sync.dma_start(beta_rep[:], gb[1:2, :].to_broadcast([P, DM]))

            for tb in range(NTB):
                tsl = slice(tb * TB, (tb + 1) * TB)
                qf_t = xin.tile([P, NCH, TB], _BF, tag="qf")
                sk_t = xin.tile([P, NCH, TB], _BF, tag="sk")
                skv_t = xin.tile([P, NCH, TB], _BF, tag="skv")
                nc.sync.dma_start(qf_t[:], qf_view[:, :, tsl])
                nc.sync.dma_start(sk_t[:], sk_view[:, :, tsl])
                nc.sync.dma_start(skv_t[:], skv_view[:, :, tsl])

                # denominators for all 16 heads: dn[h, t]
                dn = psdn.tile([H, TB], _FP, tag="dn")
                for ci in range(NCH):
                    p1 = ops.tile([P, TB], _BF, tag="p1")
                    nc.vector.tensor_tensor(p1[:], sk_t[:, ci], qf_t[:, ci], _ALU.mult)
                    nc.tensor.matmul(
                        dn[:], hm_sb[:, ci], p1[:],
                        start=(ci == 0), stop=(ci == NCH - 1),
                    )
                dn_sb = att.tile([H, TB], _FP, tag="dnsb")
                nc.scalar.activation(dn_sb[:], dn[:], _ACTF.Copy, bias=EPS_ATTN)
                rc = att.tile([H, TB], _BF, tag="rc")
                with nc.allow_low_precision(reason="bf16 recip feeds bf16 matmul"):
                    nc.vector.reciprocal(rc[:], dn_sb[:])

                # A[ci] = (qf * skv) * recip(dn)  (recip broadcast over head dims)
                a_tiles = []
                for ci in range(NCH):
                    rep = psrep.tile([P, TB], _FP, tag="rep")
                    nc.tensor.matmul(rep[:], hmT_sb[:, ci], rc[:], start=True, stop=True)
                    rep_sb = ops.tile([P, TB], _BF, tag="repsb")
                    nc.scalar.activation(rep_sb[:], rep[:], _ACTF.Copy)
                    p2 = ops.tile([P, TB], _BF, tag="p2")
                    nc.vector.tensor_tensor(
                        p2[:], skv_t[:, ci], qf_t[:, ci], _ALU.mult
                    )
                    a_t = apool.tile([P, TB], _BF, tag=f"a{ci}", name=f"a{ci}")
                    nc.vector.tensor_tensor(a_t[:], p2[:], rep_sb[:], _ALU.mult)
                    a_tiles.append(a_t)

                # Wo projection + residual + LayerNorm, per 128-row subtile
                for s4 in range(TB // P):
                    row0 = tb * TB + s4 * P
                    ssl = slice(s4 * P, (s4 + 1) * P)
                    qrow = xin.tile([P, DM], _FP, tag="qrow")
                    nc.sync.dma_start(qrow[:], qrows[row0:row0 + P, :])
                    x_sb = att.tile([P, DM], _FP, tag="x")
                    xs = att.tile([P, 2], _FP, tag="xs")
                    for mb in range(DM // TB):
                        msl = slice(mb * TB, (mb + 1) * TB)
                        ao = psao.tile([P, TB], _FP, tag="ao")
                        for ci in range(NCH):
                            nc.tensor.matmul(
                                ao[:], a_tiles[ci][:, ssl], wo_sb[:, ci, msl],
                                start=(ci == 0), stop=(ci == NCH - 1),
                            )
                        nc.vector.scalar_tensor_tensor(
                            x_sb[:, msl], ao[:], 0.0, qrow[:, msl],
                            _ALU.add, _ALU.add, accum_out=xs[:, mb:mb + 1],
                        )
                    # LayerNorm stats from running sums: ACT supplies sum(x^2)
                    xsq = att.tile([P, DM], _BF, tag="xsq")
                    sq = att.tile([P, 1], _FP, tag="sq")
                    nc.scalar.activation(
                        xsq[:], x_sb[:], _ACTF.Square, accum_out=sq[:, 0:1]
                    )
                    mv = att.tile([P, 2], _FP, tag="mv")
                    # mv0 = mean, mv1 = E[x^2]
                    nc.vector.tensor_tensor(mv[:, 0:1], xs[:, 0:1], xs[:, 1:2], _ALU.add)
                    nc.vector.tensor_scalar_mul(mv[:, 0:1], mv[:, 0:1], 1.0 / DM)
                    nc.vector.tensor_scalar_mul(mv[:, 1:2], sq[:, 0:1], 1.0 / DM)
                    # var = E[x^2] - mean^2
                    var = att.tile([P, 1], _FP, tag="var")
                    nc.vector.scalar_tensor_tensor(
                        var[:], mv[:, 0:1], -1.0, mv[:, 0:1], _ALU.mult, _ALU.mult
                    )
                    nc.vector.tensor_tensor(var[:], var[:], mv[:, 1:2], _ALU.add)
                    rstd = att.tile([P, 1], _FP, tag="rstd")
                    nc.scalar.activation(
                        rstd[:], var[:, 0:1], _ACTF.Sqrt, bias=eps_sb[:, 0:1]
                    )
                    nc.vector.reciprocal(rstd[:], rstd[:])
                    y = att.tile([P, DM], _FP, tag="y")
                    if trivial_gb:
                        # y = Identity(x * rstd + (-mean*rstd)) on the idle ACT
                        nmr = att.tile([P, 1], _FP, tag="nmr")
                        nc.vector.scalar_tensor_tensor(
                            nmr[:], mv[:, 0:1], -1.0, rstd[:], _ALU.mult, _ALU.mult
                        )
                        nc.scalar.activation(
                            y[:], x_sb[:], _ACTF.Identity,
                            bias=nmr[:, 0:1], scale=rstd[:, 0:1],
                        )
                    else:
                        nc.vector.tensor_scalar(
                            y[:], x_sb[:], mv[:, 0:1], rstd[:],
                            _ALU.subtract, _ALU.mult,
                        )
                        nc.gpsimd.tensor_tensor(y[:], y[:], gamma_rep[:], _ALU.mult)
                        nc.gpsimd.tensor_tensor(y[:], y[:], beta_rep[:], _ALU.add)
                    nc.sync.dma_start(out[row0:row0 + P, :], y[:])
    return nc


# --------------------------------------------------------------------------
# Host orchestration
# --------------------------------------------------------------------------
_cache = {}


def _consts():
    if "hm" in _cache:
        return
    hm = np.zeros((P, NCH, H), BF16)
    hmT = np.zeros((H, NCH, P), BF16)
    for o in range(NCH):
        for p in range(P):
            j = o * HPC + p // D
            hm[p, o, j] = 1.0
            hmT[j, o, p] = 1.0
    _cache["hm"] = hm
    _cache["hmT"] = hmT


def _w_chunks(w):
    # (DM, DM) -> (P, NCH, DM): [p, o, c] = w[o*P + p, c]
    return np.ascontiguousarray(
        w.astype(BF16).reshape(NCH, P, DM).transpose(1, 0, 2)
    )


def _col_chunks(v):
    # (DM,) -> (P, NCH): [p, o] = v[o*P + p]
    return np.ascontiguousarray(v.astype(F32).reshape(NCH, P).T)


def kernel(**inputs):
    query = np.ascontiguousarray(np.asarray(inputs["query"], F32))
    key_in = np.asarray(inputs.get("key_in", inputs.get("key")), F32)
    value = np.asarray(inputs["value"], F32)
    Wq, Wk, Wv, Wo = (np.asarray(inputs[k], F32) for k in ("Wq", "Wk", "Wv", "Wo"))
    bq, bk, bv, bo = (np.asarray(inputs[k], F32) for k in ("bq", "bk", "bv", "bo"))
    gamma = np.asarray(inputs["gamma"], F32)
    beta = np.asarray(inputs["beta"], F32)
    trivial_gb = bool((gamma == 1.0).all() and (beta == 0.0).all())

    _consts()
    if "l1" not in _cache:
        _cache["l1"] = build_l1()
    if ("l2", trivial_gb) not in _cache:
        _cache[("l2", trivial_gb)] = build_l2(trivial_gb)
    nc1 = _cache["l1"]
    nc2 = _cache[("l2", trivial_gb)]

    wq_c, wk_c, wv_c, wo_c = map(_w_chunks, (Wq, Wk, Wv, Wo))
    bqkv = np.ascontiguousarray(
        np.concatenate([_col_chunks(bq), _col_chunks(bk), _col_chunks(bv)], axis=1)
    )
    gb = np.ascontiguousarray(np.stack([gamma, beta]).astype(F32))

    core_ids = list(range(NCORES))
    in_maps1 = []
    for c in core_ids:
        b, h = c // 2, c % 2
        rows = slice(h * LH, (h + 1) * LH)
        in_maps1.append({
            "qT": np.ascontiguousarray(query[b, rows, :].astype(BF16).T),
            "kT": np.ascontiguousarray(key_in[b, rows, :].astype(BF16).T),
            "vT": np.ascontiguousarray(value[b, rows, :].astype(BF16).T),
            "wq": wq_c, "wk": wk_c, "wv": wv_c, "bqkv": bqkv,
        })

    r1 = run_bass_kernel_spmd(nc1, in_maps1, core_ids, trace=TRACE)
    if TRACE:
        LAST_PROFILE["l1_ns"] = r1.exec_time_ns
        LAST_PROFILE["l1_json"] = r1.profile_json

    in_maps2 = []
    for c in core_ids:
        b, h = c // 2, c % 2
        rows = slice(h * LH, (h + 1) * LH)
        sk_arr = np.asarray(r1.results[c]["sk"])
        skv_arr = np.asarray(r1.results[c]["skv"])
        if h == 1:
            # fold the first-half totals into this core's cumsums
            tot_arr = np.asarray(r1.results[2 * b]["tot"], F32)  # (P, 2*NCH)
            off_k = tot_arr[:, :NCH].T.reshape(DM, 1)     # [o*P+p] = tot[p, o]
            off_kv = tot_arr[:, NCH:].T.reshape(DM, 1)
            sk_arr = (sk_arr.astype(F32) + off_k).astype(BF16)
            skv_arr = (skv_arr.astype(F32) + off_kv).astype(BF16)
        m = {
            "qf": np.asarray(r1.results[c]["qf"]),
            "sk": sk_arr,
            "skv": skv_arr,
            "qrows": np.ascontiguousarray(query[b, rows, :]) + bo,
            "wo": wo_c, "hm": _cache["hm"], "hmT": _cache["hmT"],
        }
        if not trivial_gb:
            m["gb"] = gb
        in_maps2.append(m)

    r2 = run_bass_kernel_spmd(nc2, in_maps2, core_ids, trace=TRACE)
    if TRACE:
        LAST_PROFILE["l2_ns"] = r2.exec_time_ns
        LAST_PROFILE["l2_json"] = r2.profile_json

    out = np.empty((B, L, DM), F32)
    for c in core_ids:
        b, h = c // 2, c % 2
        out[b, h * LH:(h + 1) * LH, :] = np.asarray(r2.results[c]["out"], F32)
    return out

